# revision 42
# baseline (speedup 1.0000x reference)
"""Batch Graph-Attention layer (GAT, eval mode) on 8 Trainium2 NeuronCores.

Math per graph b (reference):
    Wh = h @ W                         (N=1024, Fo=64)
    f1 = Wh @ a1 ; f2 = Wh @ a2        (N,)
    e[i,j]   = leakyrelu(f1[i]+f2[j], 0.01)
    att      = softmax(e, axis=j)
    out      = elu(att @ Wh)

Device algorithm (per graph), avoiding any O(N^2) transcendentals:
    exp(lrelu(x)) == max(exp(x), exp(0.01x))          (exact for slope in (0,1))
    exp(f1[i]+f2[j]) == g1[i]*g2[j]  (rank-1)          g = exp(f), hh = exp(0.01 f)
 => expe[j,i] = g2[j] * max(g1[i], r[j]*hh1[i]),       r = exp(-0.99 f2)
    tmp_c = hh1b * r_c   (tensor_scalar, spread over GpSimd/DVE/ACT)
    v_c   = max(tmp_c, g1b)                            (DVE bf16 tensor_tensor)
    numer.T[o,i] & rowsum[i] via PE:  lhsT = [Wh*g2 | g2] (65 cols), rhs = v
    out[i,:] = elu(numer[i,:]/rowsum[i])

Sharding: batch dim 16 -> 8 cores x 2 graphs (pure data parallel, no comms).
Emission is phase-interleaved (A0 A1 B0 B1 C0 C1) so the two graphs pipeline
across engines (each engine executes its stream in order).
"""

import numpy as np

import concourse.bass as bass
import concourse.mybir as mybir
import concourse.tile as tile
from concourse import bacc

F32 = mybir.dt.float32
BF16 = mybir.dt.bfloat16
AF = mybir.ActivationFunctionType
OP = mybir.AluOpType

B_PER_CORE = 2
N = 1024
F_IN = 128
F_OUT = 64
C = N // 128  # 8 chunks of 128 rows
NEG_SLOPE = 0.01

LAST_PERF = {}


def build_bass():
    nc = bacc.Bacc("TRN2", target_bir_lowering=False, debug=False)

    h_d = nc.dram_tensor("h", [B_PER_CORE, N, F_IN], F32, kind="ExternalInput")
    w_d = nc.dram_tensor("W", [B_PER_CORE, F_IN, F_OUT], F32, kind="ExternalInput")
    a_d = nc.dram_tensor("a", [B_PER_CORE, 1, 2 * F_OUT, 1], F32, kind="ExternalInput")
    i_d = nc.dram_tensor("ident", [128, 128], F32, kind="ExternalInput")
    o_d = nc.dram_tensor("out", [B_PER_CORE, N, F_OUT], F32, kind="ExternalOutput")

    with tile.TileContext(nc) as tc:
        with (
            tc.tile_pool(name="singles", bufs=1) as singles,
            tc.tile_pool(name="hin", bufs=2) as hin_pool,
            tc.tile_pool(name="ht", bufs=2) as ht_pool,
            tc.tile_pool(name="small", bufs=2) as small_pool,
            tc.tile_pool(name="bcast", bufs=2) as bcast_pool,
            tc.tile_pool(name="v", bufs=10) as v_pool,
            tc.tile_pool(name="tmp", bufs=10) as tmp_pool,
            tc.tile_pool(name="tail", bufs=2) as tail_pool,
            tc.tile_pool(name="ps", bufs=7, space="PSUM") as ps,
            tc.tile_pool(name="pswarm", bufs=1, space="PSUM") as pswarm,
        ):
            ident = singles.tile([128, 128], F32)
            nc.sync.dma_start(out=ident[:], in_=i_d[:])
            zeros = singles.tile([128, 128], F32)
            nc.vector.memset(zeros[:], 0.0)

            # PE warm-up: dense junk matmuls during the DMA/preamble window so
            # the HAM clock gate reaches 8/8 before the real PE work arrives.
            warm_ps = pswarm.tile([128, 128], F32, tag="warm")
            for _ in range(16):
                nc.tensor.matmul(warm_ps[:], zeros[:], zeros[:])

            st = [dict() for _ in range(B_PER_CORE)]

            def stage_a(b):
                """Loads, h/W transposes, Wh+f2, exps of vectors, whg, f1b,
                broadcast exps."""
                s = st[b]
                h_sb = hin_pool.tile([128, C, F_IN], F32, tag="h")
                h_src = h_d[b].rearrange("(c p) f -> p c f", p=128)
                nc.sync.dma_start(out=h_sb[:, 0:4, :], in_=h_src[:, 0:4, :])
                nc.sync.dma_start(out=h_sb[:, 4:8, :], in_=h_src[:, 4:8, :])
                wext = small_pool.tile([128, F_OUT + 1], F32, tag="wext")
                nc.sync.dma_start(out=wext[:, 0:F_OUT], in_=w_d[b])
                apair = small_pool.tile([F_OUT, 2], F32, tag="apair")
                nc.sync.dma_start(
                    out=apair[:],
                    in_=a_d[b, 0, :, 0].rearrange("(two o) -> o two", two=2),
                )

                # W.T, then wa12 = W @ [a1|a2]  (shape [F_IN, 2])
                wt_ps = ps.tile([F_OUT, 128], F32, tag="ps")
                nc.tensor.transpose(wt_ps[:], wext[:, 0:F_OUT], ident[:])
                wt_sb = small_pool.tile([F_OUT, 128], F32, tag="wt")
                nc.vector.tensor_copy(wt_sb[:], wt_ps[:])

                wa_ps = ps.tile([128, 2], F32, tag="ps")
                nc.tensor.matmul(wa_ps[:], wt_sb[:], apair[:])
                wa_sb = small_pool.tile([128, 2], F32, tag="wa")
                nc.vector.tensor_copy(wa_sb[:], wa_ps[:])
                # wa2 becomes column 64 of the Wh matmul rhs -> f2 per chunk
                nc.vector.tensor_copy(wext[:, F_OUT : F_OUT + 1], wa_sb[:, 1:2])
                # broadcast wa1 along free dim -> lhsT for the f1-broadcast mm
                wa1b = small_pool.tile([128, 128], F32, tag="wa1b")
                nc.vector.tensor_scalar(
                    wa1b[:], zeros[:], wa_sb[:, 0:1], None, op0=OP.add
                )

                # transpose h -> ht [f, n]
                ht_sb = ht_pool.tile([128, N], F32, tag="ht")
                for half in range(2):
                    pt = ps.tile([128, 4, 128], F32, tag="ps")
                    for q in range(4):
                        c = half * 4 + q
                        nc.tensor.transpose(pt[:, q, :], h_sb[:, c, :], ident[:])
                    dst = ht_sb[:, half * 512 : (half + 1) * 512]
                    if half == 0:
                        nc.scalar.copy(dst, pt[:])
                    else:
                        nc.vector.tensor_copy(dst, pt[:])

                # Wh chunks (+f2 col): out[:, c, 0:64] = Wh_c, [:, c, 64] = f2_c
                pwh = []
                for half in range(2):
                    p = ps.tile([128, 4, F_OUT + 1], F32, tag="ps")
                    pwh.append(p)
                    for q in range(4):
                        c = half * 4 + q
                        nc.tensor.matmul(
                            p[:, q, :], ht_sb[:, c * 128 : (c + 1) * 128], wext[:]
                        )

                # r = exp(-0.99 f2) per chunk col (B-phase critical: emit first)
                g2 = small_pool.tile([128, C], F32, tag="g2")
                r_sb = small_pool.tile([128, C], F32, tag="r")
                for half in range(2):
                    sl = slice(half * 4, (half + 1) * 4)
                    nc.scalar.activation(
                        r_sb[:, sl], pwh[half][:, :, F_OUT], AF.Exp,
                        scale=-(1.0 - NEG_SLOPE),
                    )

                # f1 broadcast to all 128 partitions via PE
                pf1b = []
                for half in range(2):
                    p = ps.tile([128, 512], F32, tag="ps")
                    pf1b.append(p)
                    nc.tensor.matmul(
                        p[:], wa1b[:], ht_sb[:, half * 512 : (half + 1) * 512]
                    )

                # hh1b = exp(0.01 f1b), g1b = exp(f1b)  [128, 1024]
                g1b = bcast_pool.tile([128, N], BF16, tag="g1b")
                hh1b = bcast_pool.tile([128, N], BF16, tag="hh1b")
                for half in range(2):
                    sl = slice(half * 512, (half + 1) * 512)
                    nc.scalar.activation(
                        hh1b[:, sl], pf1b[half][:], AF.Exp, scale=NEG_SLOPE
                    )
                for half in range(2):
                    sl = slice(half * 512, (half + 1) * 512)
                    nc.scalar.activation(g1b[:, sl], pf1b[half][:], AF.Exp)

                # g2 = exp(f2), then whg = [Wh * g2 | g2 | 0] (needed only by
                # the final matmuls -> late). 66 cols so the g2 column lands
                # on a 4-byte-aligned bf16 PAIR (strided single-bf16 writes
                # cost a per-element RMW).
                for half in range(2):
                    sl = slice(half * 4, (half + 1) * 4)
                    nc.scalar.activation(g2[:, sl], pwh[half][:, :, F_OUT], AF.Exp)
                whg = small_pool.tile([128, C, F_OUT + 2], BF16, tag="whg")
                for c in range(C):
                    nc.scalar.activation(
                        whg[:, c, 0:F_OUT],
                        pwh[c // 4][:, c % 4, 0:F_OUT],
                        AF.Copy,
                        scale=g2[:, c : c + 1],
                    )
                g2z = small_pool.tile([128, C, 2], F32, tag="g2z")
                nc.vector.memset(g2z[:], 0.0)
                nc.vector.tensor_copy(g2z[:, :, 0], g2[:])
                nc.vector.tensor_copy(whg[:, :, F_OUT : F_OUT + 2], g2z[:])
                # tmp_c = hh1b * r_c hoisted here: VE has slack in phase A, so
                # the B-phase DVE cadence is only the max op per chunk.
                tmps = []
                for c in range(C):
                    tmp = tmp_pool.tile([128, N], BF16, tag="tmp")
                    tmps.append(tmp)
                    nc.vector.tensor_scalar(
                        tmp[:], hh1b[:], r_sb[:, c : c + 1], None, op0=OP.mult
                    )

                s.update(
                    ht=ht_sb, g2=g2, r=r_sb, whg=whg, g1b=g1b,
                    hh1b=hh1b, pf1b=pf1b, tmps=tmps,
                )

            def stage_b(b):
                """Per chunk: tmp = hh1b*r_c (GpSimd/DVE), v = max(tmp, g1b)
                (DVE bf16 2x), then the accumulating final matmuls."""
                s = st[b]
                v_tiles = []
                for c in range(C):
                    v = v_pool.tile([128, N], BF16, tag="v")
                    v_tiles.append(v)
                    nc.vector.tensor_tensor(
                        v[:], s["tmps"][c][:], s["g1b"][:], op=OP.max
                    )

                phpT = []
                for half in range(2):
                    p = ps.tile([F_OUT + 1, 512], F32, tag="ps")
                    phpT.append(p)
                    for c in range(C):
                        nc.tensor.matmul(
                            p[:],
                            s["whg"][:, c, 0 : F_OUT + 1],
                            v_tiles[c][:, half * 512 : (half + 1) * 512],
                            start=(c == 0),
                            stop=(c == C - 1),
                        )
                s["phpT"] = phpT

            def stage_c(b):
                """Transpose numer.T back, normalize, ELU, store."""
                s = st[b]
                hpT_sb = tail_pool.tile([F_OUT + 1, N], F32, tag="hpT")
                for half in range(2):
                    dst = hpT_sb[:, half * 512 : (half + 1) * 512]
                    if half == 0:
                        nc.scalar.copy(dst, s["phpT"][half][:])
                    else:
                        nc.vector.tensor_copy(dst, s["phpT"][half][:])
                php = []
                for half in range(2):
                    p = ps.tile([128, 4, F_OUT + 1], F32, tag="ps")
                    php.append(p)
                    for q in range(4):
                        c = half * 4 + q
                        nc.tensor.transpose(
                            p[:, q, :],
                            hpT_sb[:, c * 128 : (c + 1) * 128],
                            ident[: F_OUT + 1, : F_OUT + 1],
                        )

                rz = small_pool.tile([128, C], F32, tag="rz")
                for half in range(2):
                    sl = slice(half * 4, (half + 1) * 4)
                    nc.vector.reciprocal(rz[:, sl], php[half][:, :, F_OUT])
                hp = tail_pool.tile([128, C, F_OUT], F32, tag="hp")
                for c in range(C):
                    if c % 2 == 0:
                        nc.vector.tensor_scalar(
                            hp[:, c, :],
                            php[c // 4][:, c % 4, 0:F_OUT],
                            rz[:, c : c + 1],
                            None,
                            op0=OP.mult,
                        )
                    else:
                        nc.scalar.activation(
                            hp[:, c, :],
                            php[c // 4][:, c % 4, 0:F_OUT],
                            AF.Copy,
                            scale=rz[:, c : c + 1],
                        )
                # elu(x) = max(x,0) - relu(1 - exp(x))
                te = tail_pool.tile([128, C, F_OUT], F32, tag="te")
                nc.scalar.activation(te[:], hp[:], AF.Exp)
                rt = tail_pool.tile([128, C, F_OUT], F32, tag="rt")
                nc.scalar.activation(rt[:], te[:], AF.Relu, scale=-1.0, bias=1.0)
                osb = tail_pool.tile([128, C, F_OUT], F32, tag="osb")
                nc.vector.scalar_tensor_tensor(
                    osb[:], hp[:], 0.0, rt[:], op0=OP.max, op1=OP.subtract
                )
                nc.sync.dma_start(
                    out=o_d[b].rearrange("(c p) o -> p c o", p=128), in_=osb[:]
                )

            stage_a(0)
            stage_a(1)
            stage_b(0)
            stage_b(1)
            stage_c(0)
            stage_c(1)

    nc.compile()
    return nc


def kernel(h: np.ndarray, W: np.ndarray, a: np.ndarray, _trace: bool = False):
    from concourse.bass_utils import run_bass_kernel_spmd

    n_cores = 8
    nc = build_bass()
    ident = np.eye(128, dtype=np.float32)
    in_maps = []
    for i in range(n_cores):
        sl = slice(i * B_PER_CORE, (i + 1) * B_PER_CORE)
        in_maps.append(
            {
                "h": np.ascontiguousarray(h[sl]),
                "W": np.ascontiguousarray(W[sl]),
                "a": np.ascontiguousarray(a[sl]),
                "ident": ident,
            }
        )
    res = run_bass_kernel_spmd(
        nc, in_maps, core_ids=list(range(n_cores)), trace=_trace
    )
    LAST_PERF.clear()
    LAST_PERF.update(
        {
            "exec_time_ns": res.exec_time_ns,
            "mean_exec_time_ns": res.mean_exec_time_ns,
            "trace": res.instructions_and_trace[1]
            if res.instructions_and_trace
            else None,
        }
    )
    return np.concatenate([r["out"] for r in res.results], axis=0)


# revision 46
# speedup vs baseline: 1.0857x; 1.0857x over previous
"""Batch Graph-Attention layer (GAT, eval mode) on 8 Trainium2 NeuronCores.

Math per graph b (reference):
    Wh = h @ W                         (N=1024, Fo=64)
    f1 = Wh @ a1 ; f2 = Wh @ a2        (N,)
    e[i,j]   = leakyrelu(f1[i]+f2[j], 0.01)
    att      = softmax(e, axis=j)
    out      = elu(att @ Wh)

Device algorithm (per graph), avoiding any O(N^2) transcendentals:
    exp(lrelu(x)) == max(exp(x), exp(0.01x))          (exact for slope in (0,1))
    exp(f1[i]+f2[j]) == g1[i]*g2[j]  (rank-1)          g = exp(f), hh = exp(0.01 f)
 => expe[j,i] = g2[j] * max(g1[i], r[j]*hh1[i]),       r = exp(-0.99 f2)
    tmp_c = hh1b * r_c   (tensor_scalar, spread over GpSimd/DVE/ACT)
    v_c   = max(tmp_c, g1b)                            (DVE bf16 tensor_tensor)
    numer.T[o,i] & rowsum[i] via PE:  lhsT = [Wh*g2 | g2] (65 cols), rhs = v
    out[i,:] = elu(numer[i,:]/rowsum[i])

Sharding: batch dim 16 -> 8 cores x 2 graphs (pure data parallel, no comms).
Emission is phase-interleaved (A0 A1 B0 B1 C0 C1) so the two graphs pipeline
across engines (each engine executes its stream in order).
"""

import numpy as np

import concourse.bass as bass
import concourse.mybir as mybir
import concourse.tile as tile
from concourse import bacc

F32 = mybir.dt.float32
BF16 = mybir.dt.bfloat16
AF = mybir.ActivationFunctionType
OP = mybir.AluOpType

B_PER_CORE = 2
N = 1024
F_IN = 128
F_OUT = 64
C = N // 128  # 8 chunks of 128 rows
NEG_SLOPE = 0.01

LAST_PERF = {}


def build_bass():
    nc = bacc.Bacc("TRN2", target_bir_lowering=False, debug=False)

    h_d = nc.dram_tensor("h", [B_PER_CORE, N, F_IN], F32, kind="ExternalInput")
    w_d = nc.dram_tensor("W", [B_PER_CORE, F_IN, F_OUT], F32, kind="ExternalInput")
    a_d = nc.dram_tensor("a", [B_PER_CORE, 1, 2 * F_OUT, 1], F32, kind="ExternalInput")
    i_d = nc.dram_tensor("ident", [128, 128], F32, kind="ExternalInput")
    o_d = nc.dram_tensor("out", [B_PER_CORE, N, F_OUT], F32, kind="ExternalOutput")

    with tile.TileContext(nc) as tc:
        with (
            tc.tile_pool(name="singles", bufs=1) as singles,
            tc.tile_pool(name="hin", bufs=2) as hin_pool,
            tc.tile_pool(name="ht", bufs=2) as ht_pool,
            tc.tile_pool(name="small", bufs=2) as small_pool,
            tc.tile_pool(name="bcast", bufs=2) as bcast_pool,
            tc.tile_pool(name="v", bufs=10) as v_pool,
            tc.tile_pool(name="tmp", bufs=3) as tmp_pool,
            tc.tile_pool(name="tail", bufs=2) as tail_pool,
            tc.tile_pool(name="ps", bufs=7, space="PSUM") as ps,
            tc.tile_pool(name="pswarm", bufs=1, space="PSUM") as pswarm,
        ):
            ident = singles.tile([128, 128], F32)
            nc.sync.dma_start(out=ident[:], in_=i_d[:])
            zeros = singles.tile([128, 128], F32)
            nc.vector.memset(zeros[:], 0.0)

            # PE warm-up: dense junk matmuls during the DMA/preamble window so
            # the HAM clock gate reaches 8/8 before the real PE work arrives.
            warm_ps = pswarm.tile([128, 128], F32, tag="warm")
            for _ in range(16):
                nc.tensor.matmul(warm_ps[:], zeros[:], zeros[:])

            st = [dict() for _ in range(B_PER_CORE)]

            def stage_a(b):
                """Loads, h/W transposes, Wh+f2, exps of vectors, whg, f1b,
                broadcast exps."""
                s = st[b]
                h_sb = hin_pool.tile([128, C, F_IN], F32, tag="h")
                h_src = h_d[b].rearrange("(c p) f -> p c f", p=128)
                for q4 in range(4):
                    nc.sync.dma_start(
                        out=h_sb[:, 2 * q4 : 2 * q4 + 2, :],
                        in_=h_src[:, 2 * q4 : 2 * q4 + 2, :],
                    )
                wext = small_pool.tile([128, F_OUT + 1], F32, tag="wext")
                nc.sync.dma_start(out=wext[:, 0:F_OUT], in_=w_d[b])
                apair = small_pool.tile([F_OUT, 2], F32, tag="apair")
                nc.sync.dma_start(
                    out=apair[:],
                    in_=a_d[b, 0, :, 0].rearrange("(two o) -> o two", two=2),
                )

                # W.T, then wa12 = W @ [a1|a2]  (shape [F_IN, 2])
                wt_ps = ps.tile([F_OUT, 128], F32, tag="ps")
                nc.tensor.transpose(wt_ps[:], wext[:, 0:F_OUT], ident[:])
                wt_sb = small_pool.tile([F_OUT, 128], F32, tag="wt")
                nc.vector.tensor_copy(wt_sb[:], wt_ps[:])

                wa_ps = ps.tile([128, 2], F32, tag="ps")
                nc.tensor.matmul(wa_ps[:], wt_sb[:], apair[:])
                wa_sb = small_pool.tile([128, 2], F32, tag="wa")
                nc.vector.tensor_copy(wa_sb[:], wa_ps[:])
                # wa2 becomes column 64 of the Wh matmul rhs -> f2 per chunk
                nc.vector.tensor_copy(wext[:, F_OUT : F_OUT + 1], wa_sb[:, 1:2])
                # broadcast wa1 along free dim -> lhsT for the f1-broadcast mm
                wa1b = small_pool.tile([128, 128], F32, tag="wa1b")
                nc.vector.tensor_scalar(
                    wa1b[:], zeros[:], wa_sb[:, 0:1], None, op0=OP.add
                )

                # transpose h -> ht [f, n]
                ht_sb = ht_pool.tile([128, N], F32, tag="ht")
                for half in range(2):
                    pt = ps.tile([128, 4, 128], F32, tag="ps")
                    for q in range(4):
                        c = half * 4 + q
                        nc.tensor.transpose(pt[:, q, :], h_sb[:, c, :], ident[:])
                    dst = ht_sb[:, half * 512 : (half + 1) * 512]
                    if half == 0:
                        nc.scalar.copy(dst, pt[:])
                    else:
                        nc.vector.tensor_copy(dst, pt[:])

                # Wh chunks (+f2 col): out[:, c, 0:64] = Wh_c, [:, c, 64] = f2_c
                pwh = []
                for half in range(2):
                    p = ps.tile([128, 4, F_OUT + 1], F32, tag="ps")
                    pwh.append(p)
                    for q in range(4):
                        c = half * 4 + q
                        nc.tensor.matmul(
                            p[:, q, :], ht_sb[:, c * 128 : (c + 1) * 128], wext[:]
                        )

                # r = exp(-0.99 f2) per chunk col (B-phase critical: emit first)
                g2 = small_pool.tile([128, C], F32, tag="g2")
                r_sb = small_pool.tile([128, C], F32, tag="r")
                for half in range(2):
                    sl = slice(half * 4, (half + 1) * 4)
                    nc.scalar.activation(
                        r_sb[:, sl], pwh[half][:, :, F_OUT], AF.Exp,
                        scale=-(1.0 - NEG_SLOPE),
                    )

                # f1 broadcast to all 128 partitions via PE
                pf1b = []
                for half in range(2):
                    p = ps.tile([128, 512], F32, tag="ps")
                    pf1b.append(p)
                    nc.tensor.matmul(
                        p[:], wa1b[:], ht_sb[:, half * 512 : (half + 1) * 512]
                    )

                # hh1b = exp(0.01 f1b), g1b = exp(f1b)  [128, 1024]
                g1b = bcast_pool.tile([128, N], BF16, tag="g1b")
                hh1b = bcast_pool.tile([128, N], BF16, tag="hh1b")
                for half in range(2):
                    sl = slice(half * 512, (half + 1) * 512)
                    nc.scalar.activation(
                        hh1b[:, sl], pf1b[half][:], AF.Exp, scale=NEG_SLOPE
                    )
                for half in range(2):
                    sl = slice(half * 512, (half + 1) * 512)
                    nc.scalar.activation(g1b[:, sl], pf1b[half][:], AF.Exp)

                # g2 = exp(f2), then whg = [Wh * g2 | g2 | 0] (needed only by
                # the final matmuls -> late). 66 cols so the g2 column lands
                # on a 4-byte-aligned bf16 PAIR (strided single-bf16 writes
                # cost a per-element RMW).
                for half in range(2):
                    sl = slice(half * 4, (half + 1) * 4)
                    nc.scalar.activation(g2[:, sl], pwh[half][:, :, F_OUT], AF.Exp)
                whg = small_pool.tile([128, C, F_OUT + 2], BF16, tag="whg")
                for c in range(C):
                    nc.scalar.activation(
                        whg[:, c, 0:F_OUT],
                        pwh[c // 4][:, c % 4, 0:F_OUT],
                        AF.Copy,
                        scale=g2[:, c : c + 1],
                    )
                g2z = small_pool.tile([128, C, 2], F32, tag="g2z")
                nc.vector.memset(g2z[:], 0.0)
                nc.vector.tensor_copy(g2z[:, :, 0], g2[:])
                nc.vector.tensor_copy(whg[:, :, F_OUT : F_OUT + 2], g2z[:])
                s.update(
                    ht=ht_sb, g2=g2, r=r_sb, whg=whg, g1b=g1b,
                    hh1b=hh1b, pf1b=pf1b,
                )

            def stage_b(b):
                """Per chunk: tmp = hh1b*r_c (GpSimd/DVE), v = max(tmp, g1b)
                (DVE bf16 2x), then the accumulating final matmuls."""
                s = st[b]
                v_tiles = []
                for c in range(C):
                    tmp = tmp_pool.tile([128, N], BF16, tag="tmp")
                    nc.vector.tensor_scalar(
                        tmp[:], s["hh1b"][:], s["r"][:, c : c + 1], None,
                        op0=OP.mult,
                    )
                    v = v_pool.tile([128, N], BF16, tag="v")
                    v_tiles.append(v)
                    nc.vector.tensor_tensor(v[:], tmp[:], s["g1b"][:], op=OP.max)

                phpT = []
                for half in range(2):
                    p = ps.tile([F_OUT + 1, 512], F32, tag="ps")
                    phpT.append(p)
                    for c in range(C):
                        nc.tensor.matmul(
                            p[:],
                            s["whg"][:, c, 0 : F_OUT + 1],
                            v_tiles[c][:, half * 512 : (half + 1) * 512],
                            start=(c == 0),
                            stop=(c == C - 1),
                        )
                s["phpT"] = phpT

            def stage_c(b):
                """Transpose numer.T back, normalize, ELU, store."""
                s = st[b]
                hpT_sb = tail_pool.tile([F_OUT + 1, N], F32, tag="hpT")
                for half in range(2):
                    dst = hpT_sb[:, half * 512 : (half + 1) * 512]
                    if half == 0:
                        nc.scalar.copy(dst, s["phpT"][half][:])
                    else:
                        nc.vector.tensor_copy(dst, s["phpT"][half][:])
                php = []
                for half in range(2):
                    p = ps.tile([128, 4, F_OUT + 1], F32, tag="ps")
                    php.append(p)
                    for q in range(4):
                        c = half * 4 + q
                        nc.tensor.transpose(
                            p[:, q, :],
                            hpT_sb[:, c * 128 : (c + 1) * 128],
                            ident[: F_OUT + 1, : F_OUT + 1],
                        )

                rz = small_pool.tile([128, C], F32, tag="rz")
                for half in range(2):
                    sl = slice(half * 4, (half + 1) * 4)
                    nc.vector.reciprocal(rz[:, sl], php[half][:, :, F_OUT])
                hp = tail_pool.tile([128, C, F_OUT], F32, tag="hp")
                for c in range(C):
                    if c % 2 == 0:
                        nc.vector.tensor_scalar(
                            hp[:, c, :],
                            php[c // 4][:, c % 4, 0:F_OUT],
                            rz[:, c : c + 1],
                            None,
                            op0=OP.mult,
                        )
                    else:
                        nc.scalar.activation(
                            hp[:, c, :],
                            php[c // 4][:, c % 4, 0:F_OUT],
                            AF.Copy,
                            scale=rz[:, c : c + 1],
                        )
                # elu(x) = max(x,0) - relu(1 - exp(x))
                te = tail_pool.tile([128, C, F_OUT], F32, tag="te")
                nc.scalar.activation(te[:], hp[:], AF.Exp)
                rt = tail_pool.tile([128, C, F_OUT], F32, tag="rt")
                nc.scalar.activation(rt[:], te[:], AF.Relu, scale=-1.0, bias=1.0)
                osb = tail_pool.tile([128, C, F_OUT], F32, tag="osb")
                nc.vector.scalar_tensor_tensor(
                    osb[:], hp[:], 0.0, rt[:], op0=OP.max, op1=OP.subtract
                )
                nc.sync.dma_start(
                    out=o_d[b].rearrange("(c p) o -> p c o", p=128), in_=osb[:]
                )

            stage_a(0)
            stage_a(1)
            stage_b(0)
            stage_b(1)
            stage_c(0)
            stage_c(1)

    nc.compile()
    return nc


def kernel(h: np.ndarray, W: np.ndarray, a: np.ndarray, _trace: bool = False):
    from concourse.bass_utils import run_bass_kernel_spmd

    n_cores = 8
    nc = build_bass()
    ident = np.eye(128, dtype=np.float32)
    in_maps = []
    for i in range(n_cores):
        sl = slice(i * B_PER_CORE, (i + 1) * B_PER_CORE)
        in_maps.append(
            {
                "h": np.ascontiguousarray(h[sl]),
                "W": np.ascontiguousarray(W[sl]),
                "a": np.ascontiguousarray(a[sl]),
                "ident": ident,
            }
        )
    res = run_bass_kernel_spmd(
        nc, in_maps, core_ids=list(range(n_cores)), trace=_trace
    )
    LAST_PERF.clear()
    LAST_PERF.update(
        {
            "exec_time_ns": res.exec_time_ns,
            "mean_exec_time_ns": res.mean_exec_time_ns,
            "trace": res.instructions_and_trace[1]
            if res.instructions_and_trace
            else None,
        }
    )
    return np.concatenate([r["out"] for r in res.results], axis=0)
